# revision 6
# baseline (speedup 1.0000x reference)
"""Kalman filter predictor kernel for trn2 (8 NeuronCores, data-parallel batch shard).

Math: the reference's per-step update is a linear time-varying recurrence
    x_{t+1} = A_t x_t + B_t z_t
with A_t/B_t batch-independent.  For these inputs (F=I, H=eye(64,128),
Q/R/P0 scalar multiples of I, x0=0) every A_t is DIAGONAL with all 64
active entries equal, and every B_t is diagonal on its top 64 rows with
equal entries.  So the whole scan collapses to 64 identical independent
scalar recurrences, i.e. a single shared lower-triangular matrix
    W[t, s] = k_s * prod_{u=s+1..t} a_u        (precomputed on host in f64)
applied over the time axis:  out[b, t, i] = sum_s W[t, s] z[b, s, i].

Device work per core (256 samples): ONE [128 x 128] weight, rhs = z packed
as [t=128 partitions, 256*64 free] fp16, 32 independent N=512 matmuls into
f32 PSUM, cast-copy to fp16 SBUF (alternating Vector/Scalar engines), DMA
out.  No sequential dependency at all -> the kernel is HBM-bound:
~4 MiB in + ~4 MiB out per core.
"""

import numpy as np

N_CORES = 8
ST = 128          # state dim
PART = 128        # SBUF partitions (= T, time steps)
BS = 256          # batch per core
CHUNK_IN = 4096   # z columns per in-DMA (1 MiB fp16); all chunks prefetched
CHUNK_OUT = 2048  # out columns per out-DMA (512 KiB fp16, SWDGE/gpsimd path)
MM_N = 512        # matmul free dim (one f32 PSUM bank)

_CACHE = {}


def _precompute(F, H, Q, R, P, x, T):
    """A_t, B_t for t in [0, T) in float64, exactly mirroring the reference."""
    F = F.astype(np.float64); H = H.astype(np.float64)
    Q = Q.astype(np.float64); R = R.astype(np.float64)
    Pc = P.astype(np.float64)
    st = F.shape[0]
    As, Bs = [], []
    I = np.eye(st)
    for _ in range(T):
        Pp = F @ Pc @ F.T + Q
        S = H @ Pp @ H.T + R
        K = Pp @ H.T @ np.linalg.inv(S)
        As.append((I - K @ H) @ F)
        Bs.append(K)
        Pc = Pp - K @ H @ Pp
    return As, Bs


def _scalar_structure(As, Bs, x, OBS):
    """If every A_t is diagonal (active diag equal, inactive untouched-from-0),
    every B_t is equal-diagonal on its top OBS rows and zero below, and x0=0,
    return (a_t, k_t); else None."""
    st = As[0].shape[0]
    if np.count_nonzero(x) != 0:
        return None
    for A, B in zip(As, Bs):
        if np.count_nonzero(A - np.diag(np.diag(A))) != 0:
            return None
        d = np.diag(A)
        if np.ptp(d[:OBS]) != 0.0:
            return None
        if np.count_nonzero(B[OBS:]) != 0:
            return None
        Btop = B[:OBS, :OBS]
        if np.count_nonzero(Btop - np.diag(np.diag(Btop))) != 0:
            return None
        if np.ptp(np.diag(Btop)) != 0.0:
            return None
        if np.count_nonzero(B[:OBS, OBS:]) != 0:
            return None
    a_t = np.array([A[0, 0] for A in As])
    k_t = np.array([B[0, 0] for B in Bs])
    return a_t, k_t


def _host_fallback(feats, As, Bs, x, T, OBS):
    b = feats.shape[0]
    st = As[0].shape[0]
    z = feats.reshape(b, T, OBS).astype(np.float32)
    xs = np.broadcast_to(x.astype(np.float32), (b, st)).copy()
    out = np.empty((b, T, st), np.float32)
    for t in range(T):
        xs = xs @ As[t].astype(np.float32).T + z[:, t, :] @ Bs[t].astype(np.float32).T[:OBS]
        out[:, t, :] = xs
    return out


def _build_nc(T, free):
    import concourse.mybir as mybir
    import concourse.tile as tile
    from concourse import bacc
    from concourse.bass import ts

    f16 = mybir.dt.float16
    f32 = mybir.dt.float32

    n_in = free // CHUNK_IN
    out_per_in = CHUNK_IN // CHUNK_OUT
    mm_per_out = CHUNK_OUT // MM_N

    nc = bacc.Bacc("TRN2", target_bir_lowering=False)
    zp_d = nc.dram_tensor("zp", [PART, free], f16, kind="ExternalInput")
    w_d = nc.dram_tensor("w", [PART, PART], f16, kind="ExternalInput")
    out_d = nc.dram_tensor("out", [PART, free], f16, kind="ExternalOutput")

    with tile.TileContext(nc) as tc:
        with (
            tc.tile_pool(name="wpool", bufs=1) as wpool,
            tc.tile_pool(name="zpool", bufs=n_in) as zpool,
            tc.tile_pool(name="spool", bufs=n_in * out_per_in) as spool,
            tc.tile_pool(name="ppool", bufs=4, space="PSUM") as ppool,
        ):
            # weight on the SWDGE path so the SP sequencer only issues z loads
            wt = wpool.tile([PART, PART], f16, tag="w")
            nc.gpsimd.dma_start(out=wt[:], in_=w_d[:])
            # prefetch ALL z chunks up front, alternating the two HWDGE
            # issue paths (SP / ACT sequencers) so doorbells land sooner
            zts = []
            for c in range(n_in):
                zt = zpool.tile([PART, CHUNK_IN], f16)
                eng = nc.sync if c % 2 == 0 else nc.scalar
                eng.dma_start(
                    out=zt[:], in_=zp_d[:, c * CHUNK_IN : (c + 1) * CHUNK_IN]
                )
                zts.append(zt)
            # per out-chunk: two matmuls fill a 2-bank PSUM tile, ONE wide
            # cast-copy (alternating Vector/Scalar) evacuates both banks
            for c in range(n_in):
                for h in range(out_per_in):
                    st_t = spool.tile([PART, CHUNK_OUT], f16)
                    for j in range(mm_per_out // 2):
                        ps = ppool.tile([PART, 2 * MM_N], f32, tag="ps")
                        for k in range(2):
                            nc.tensor.matmul(
                                ps[:, ts(k, MM_N)], wt[:],
                                zts[c][:, ts(h * mm_per_out + 2 * j + k, MM_N)],
                                start=True, stop=True,
                            )
                        if j % 2 == 0:
                            nc.vector.tensor_copy(
                                out=st_t[:, ts(j, 2 * MM_N)], in_=ps[:]
                            )
                        else:
                            nc.scalar.copy(out=st_t[:, ts(j, 2 * MM_N)], in_=ps[:])
                    o0 = (c * out_per_in + h) * CHUNK_OUT
                    nc.gpsimd.dma_start(
                        out=out_d[:, o0 : o0 + CHUNK_OUT], in_=st_t[:]
                    )
    nc.finalize()
    return nc


def _prepare(F, H, Q, R, P, x, T, OBS):
    As, Bs = _precompute(F, H, Q, R, P, x, T)
    sc = _scalar_structure(As, Bs, x.astype(np.float64), OBS)
    free = BS * OBS
    if sc is None or T != PART or T * OBS != free // BS * T:
        return {"fallback": True, "As": As, "Bs": Bs}
    a_t, k_t = sc
    # W[t, s] = k_s * prod_{u=s+1..t} a_u  (lower triangular), f64 then fp16
    W = np.zeros((T, T))
    for t in range(T):
        if t:
            W[t, :t] = a_t[t] * W[t - 1, :t]
        W[t, t] = k_t[t]
    wT = np.ascontiguousarray(W.T.astype(np.float16))  # lhsT[t, t'] = W[t', t]
    nc = _build_nc(T, free)
    return {"fallback": False, "As": As, "Bs": Bs, "wT": wT, "nc": nc}


def kernel(concatenated_features, F, H, Q, R, P, x, _trace=False):
    feats = np.asarray(concatenated_features)
    F = np.asarray(F); H = np.asarray(H); Q = np.asarray(Q)
    R = np.asarray(R); P = np.asarray(P); x = np.asarray(x)
    B = feats.shape[0]
    OBS = H.shape[0]
    st = F.shape[0]
    T = (feats.shape[1] * feats.shape[2]) // OBS

    key = (F.tobytes(), H.tobytes(), Q.tobytes(), R.tobytes(), P.tobytes(),
           x.tobytes(), T, OBS)
    if key not in _CACHE:
        _CACHE[key] = _prepare(F, H, Q, R, P, x, T, OBS)
    prep = _CACHE[key]

    if prep["fallback"] or B != N_CORES * BS or OBS != 64 or T != PART:
        return _host_fallback(feats, prep["As"], prep["Bs"], x, T, OBS)

    from concourse.bass_utils import run_bass_kernel_spmd

    # pack z: [B, T, OBS] -> per-core [T, BS*OBS] fp16 (t on partitions)
    z = feats.reshape(B, T, OBS)
    in_maps = []
    for c in range(N_CORES):
        zc = z[c * BS : (c + 1) * BS]                        # [BS, T, OBS]
        zp = np.ascontiguousarray(
            zc.transpose(1, 0, 2).reshape(T, BS * OBS), dtype=np.float16
        )
        in_maps.append({"zp": zp, "w": prep["wT"]})

    res = run_bass_kernel_spmd(
        prep["nc"], in_maps, list(range(N_CORES)), trace=_trace
    )

    out = np.zeros((B, T, st), np.float32)
    for c in range(N_CORES):
        r = np.asarray(res.results[c]["out"])                # [T, BS*OBS] fp16
        out[c * BS : (c + 1) * BS, :, :OBS] = (
            r.reshape(T, BS, OBS).transpose(1, 0, 2).astype(np.float32)
        )
    if _trace:
        kernel._last_results = res
    return out
